# revision 55
# baseline (speedup 1.0000x reference)
"""Ragged masked-softmax attention-energy kernel for 8 Trainium2 NeuronCores.

Reference computation (B2=512, L=1024, E=512):
    energy = questions @ W.T + b              [B2, L, E]
    scores = energy @ weight_vec              [B2, L]
    scores[l >= len] = -inf
    out = softmax(scores, axis=1)

Algebraic facts that shape the kernel:
  * (q @ W.T + b) @ wv == q @ (W.T @ wv) + (b . wv); softmax is shift
    invariant so the (b . wv) scalar cancels. Only v = W.T @ wv (a [E]
    vector, computed on device) ever multiplies the big tensor.
  * tokens at positions >= len contribute exactly 0 to the output, so
    only ceil(len/128) 128-token tiles per row need to be loaded at all.

v3 design (PE-matmul dot products, fp8/bf16 hybrid traffic):
  * questions are packed on host TRANSPOSED per 128-token tile: 4 chunks
    of [128 E-rows x 128 tokens]. The per-token dot product with v is
    then 4 accumulating PE matmuls lhsT=[128e,128tok]^T @ v_chunk[128e,1]
    -> PSUM scores in token-major layout. Ablation: this is ~10us/pass
    of PE time vs ~73us of DMA, so the kernel is DMA-bound and bytes are
    everything.
  * rows with len >= 256 are carried in float8_e4m3 (measured worst-case
    softmax abs error 5e-3 at len 256, shrinking with length, vs the
    2e-2 budget -- long rows have small probabilities and exp() spreads
    the error); shorter rows stay bf16 (worst-case 2e-3). fp8 columns
    come first (their own DMA groups), bf16 columns after.
  * No mask tensor: host zero-fills padded tokens, so their score is
    exactly 0 and exp(0)=1; the per-column pad count is subtracted from
    the column sums (exact in fp32). Pad positions of the output are
    never read by the host-side scatter. Columns with no tile at all
    either contribute only to unused rows (segment matrices are 0) or
    are memset to -1e30 so exp gives 0.
  * Per-row softmax denominators via 0/1 segment matmuls on [1,COLS]
    column sums: PE ones-matmul column sums -> transpose -> seg matmul
    row sums -> reciprocal -> segT matmul + K=1 broadcast matmul back to
    [128,COLS] -> one DVE multiply -> DMA out.

Host side does data layout only (bin-packing, zero-fill, transpose,
fp8/bf16 cast, 0/1 indicator matrices); all arithmetic runs on device.
"""

import os
import sys

import numpy as np

if "/opt/trn_rl_repo" not in sys.path:
    sys.path.insert(0, "/opt/trn_rl_repo")

E = 512
P = 128
CH = E // P       # E-chunks per tile (contraction split for the PE)
# fp8 tiles per DMA group (1 MiB; 32-tile/2 MiB groups hard-crash the
# exec unit with NRT_EXEC_UNIT_UNRECOVERABLE) and bf16 tiles per group
TPG8 = int(os.environ.get("TPG8", "16"))
TPG16 = int(os.environ.get("TPG16", "8"))
# which engine DGEs issue the q-group loads: "sync" = all on SP HWDGE,
# "mix" = alternate SP/ACT HWDGEs, "gps" = alternate SP HWDGE / Pool SWDGE
DMAQ = os.environ.get("DMAQ", "sync")
NCORES = 8
NEG = -1.0e30
CUT = 224         # rows with len >= CUT go fp8, shorter rows bf16

_NC_CACHE = {}
LAST_RESULT = None


def _schedule(lens, n_cores):
    """Assign rows to cores (stream-aware LPT, <=128 rows/core).

    The fp8 (len>=CUT) and bf16 streams are balanced independently: the
    per-pass bytes are quantized to whole DMA groups of the max-loaded
    core per stream, so each stream's max matters separately.
    """
    k = [(int(l) + P - 1) // P for l in lens]
    rows_of = [[] for _ in range(n_cores)]
    for stream_rows in ([r for r in range(len(lens)) if lens[r] >= CUT],
                        [r for r in range(len(lens)) if lens[r] < CUT]):
        loads = [0] * n_cores
        for r in sorted(stream_rows, key=lambda r: -k[r]):
            cands = [c for c in range(n_cores) if len(rows_of[c]) < P]
            c = min(cands, key=lambda i: (loads[i], len(rows_of[i])))
            rows_of[c].append(r)
            loads[c] += k[r]
    return rows_of, k


def _pack(questions, lens, n_cores):
    import ml_dtypes

    bf16 = np.dtype(ml_dtypes.bfloat16)
    f8 = np.dtype(ml_dtypes.float8_e4m3)
    B2, L, E_ = questions.shape
    assert E_ == E
    rows_of, k = _schedule(lens, n_cores)
    cols8_of = [[(r, t) for r in rows_of[c] if lens[r] >= CUT
                 for t in range(k[r])] for c in range(n_cores)]
    cols16_of = [[(r, t) for r in rows_of[c] if lens[r] < CUT
                  for t in range(k[r])] for c in range(n_cores)]
    G8 = max(1, max(-(-len(cs) // TPG8) for cs in cols8_of))
    G16 = max(1, max(-(-len(cs) // TPG16) for cs in cols16_of))
    S8, S16 = G8 * TPG8, G16 * TPG16
    S = S8 + S16
    NB = -(-S // P)
    COLS = NB * P
    in_maps = []
    cols_meta = []
    for c in range(n_cores):
        local = {r: i for i, r in enumerate(rows_of[c])}
        # [g, e_lo, j, ch, tok] -> device column base + j*512 + ch*128 + tok
        qp8 = np.zeros((G8, P, TPG8, CH, P), np.float32)
        qp16 = np.zeros((G16, P, TPG16, CH, P), np.float32)
        padT = np.zeros((P, NB), np.float32)
        seg = np.zeros((P, COLS), np.float32)
        segT = np.zeros((P, COLS), np.float32)
        meta = []
        for qp, cols, base, tpg in ((qp8, cols8_of[c], 0, TPG8),
                                    (qp16, cols16_of[c], S8, TPG16)):
            for sl, (r, t) in enumerate(cols):
                g, j = divmod(sl, tpg)
                s = base + sl
                ntok = min(P, int(lens[r]) - t * P)
                blk = questions[r, t * P:t * P + ntok, :].T   # [512, ntok]
                qp[g, :, j, :, :ntok] = blk.reshape(CH, P, ntok).transpose(1, 0, 2)
                b_, m = divmod(s, P)
                padT[m, b_] = float(P - ntok)
                li = local[r]
                seg[m, b_ * P + li] = 1.0
                segT[li, b_ * P + m] = 1.0
                meta.append((s, r, t))
        in_maps.append({"qp8": qp8.reshape(G8, P, TPG8 * E).astype(f8),
                        "qp16": qp16.reshape(G16, P, TPG16 * E).astype(bf16),
                        "padT": padT, "seg": seg, "segT": segT})
        cols_meta.append(meta)
    return in_maps, cols_meta, G8, G16, NB


def _build_nc(G8, G16, NB, reps=1, mode="full"):
    from concourse import bacc, bass, tile

    mybir = bass.mybir
    f32 = mybir.dt.float32
    bf16 = mybir.dt.bfloat16
    f8 = mybir.dt.float8e4
    Alu = mybir.AluOpType
    ActF = mybir.ActivationFunctionType
    S8, S16 = G8 * TPG8, G16 * TPG16
    S = S8 + S16
    COLS = NB * P

    nc = bacc.Bacc("TRN2", target_bir_lowering=False, debug=False,
                   num_devices=NCORES)
    qp8 = nc.declare_dram_parameter("qp8", [G8, P, TPG8 * E], f8,
                                    isOutput=False)
    qp16 = nc.declare_dram_parameter("qp16", [G16, P, TPG16 * E], bf16,
                                     isOutput=False)
    padT = nc.declare_dram_parameter("padT", [P, NB], f32, isOutput=False)
    seg = nc.declare_dram_parameter("seg", [P, COLS], f32, isOutput=False)
    segT = nc.declare_dram_parameter("segT", [P, COLS], f32, isOutput=False)
    iden = nc.declare_dram_parameter("iden", [P, P], f32, isOutput=False)
    wm = nc.declare_dram_parameter("wm", [E, E], f32, isOutput=False)
    wv = nc.declare_dram_parameter("wv", [CH, P], f32, isOutput=False)
    # shape varies with reps/mode so the jax persistent compile cache cannot
    # alias NEFFs of different builds (the BIR is not in the HLO key)
    mid = 1 + ["full", "dma", "pe"].index(mode) \
        + 4 * ["sync", "mix", "gps"].index(DMAQ)
    nc.declare_dram_parameter("stamp", [mid, reps], f32, isOutput=False)
    probs = nc.declare_dram_parameter("probs", [P, COLS], f32, isOutput=True)

    with tile.TileContext(nc) as tc:
        with (
            tc.tile_pool(name="const", bufs=1) as const,
            tc.tile_pool(name="qpool8", bufs=12) as qpool8,
            tc.tile_pool(name="qpool16", bufs=4) as qpool16,
            tc.tile_pool(name="prpool", bufs=2) as prpool,
            tc.tile_pool(name="tpool", bufs=2) as tpool,
            tc.tile_pool(name="psb", bufs=2, space=bass.MemorySpace.PSUM) as psb,
            tc.tile_pool(name="pst", bufs=1, space=bass.MemorySpace.PSUM) as pst,
        ):
            iden_sb = const.tile([P, P], f32, tag="iden")
            nc.sync.dma_start(iden_sb[:], iden[:])
            seg_sb = const.tile([P, COLS], f32, tag="seg")
            nc.sync.dma_start(seg_sb[:], seg[:])
            segT_sb = const.tile([P, COLS], f32, tag="segT")
            nc.sync.dma_start(segT_sb[:], segT[:])
            padT_sb = const.tile([P, NB], f32, tag="padT")
            nc.sync.dma_start(padT_sb[:], padT[:])
            w_sb = const.tile([P, CH * E], f32, tag="wmat")
            for jb in range(CH):
                nc.sync.dma_start(w_sb[:, jb * E:(jb + 1) * E],
                                  wm[jb * P:(jb + 1) * P, :])
            wv4 = const.tile([CH, P], f32, tag="wv4")
            nc.sync.dma_start(wv4[:], wv[:])

            # vT[e_lo, c] = v[c*128+e_lo], v = W.T @ wv, computed on device
            wvT_ps = pst.tile([P, CH], f32, tag="su")
            nc.tensor.transpose(wvT_ps[:], wv4[:], iden_sb[0:CH, 0:CH])
            wvT_sb = const.tile([P, CH], f32, tag="wvT")
            nc.scalar.copy(wvT_sb[:], wvT_ps[:])
            vT_ps = pst.tile([P, CH], f32, tag="su")
            with tc.tile_critical():
                for c in range(CH):
                    for jb in range(CH):
                        nc.tensor.matmul(
                            vT_ps[:, c:c + 1],
                            w_sb[:, jb * E + c * P: jb * E + (c + 1) * P],
                            wvT_sb[:, jb:jb + 1],
                            start=(jb == 0), stop=(jb == CH - 1))
            vT_bf = const.tile([P, CH], bf16, tag="vTbf")
            nc.scalar.copy(vT_bf[:], vT_ps[:])
            vT_f8 = const.tile([P, CH], f8, tag="vTf8")
            nc.scalar.copy(vT_f8[:], vT_ps[:])
            ones_bf = const.tile([P, 1], bf16, tag="ones")
            nc.vector.memset(ones_bf[:], 1.0)
            ones1_sb = const.tile([1, P], f32, tag="ones1")
            nc.vector.memset(ones1_sb[:], 1.0)

            def stream_of(s):
                if s < S8:
                    return qp8, qpool8, vT_f8, f8, TPG8, 0
                return qp16, qpool16, vT_bf, bf16, TPG16, S8

            dma_engines = {"sync": (nc.sync, nc.sync),
                           "mix": (nc.sync, nc.scalar),
                           "gps": (nc.sync, nc.gpsimd)}[DMAQ]
            dma_ctr = [0]

            def q_dma(dst, src):
                dma_engines[dma_ctr[0] % 2].dma_start(dst, src)
                dma_ctr[0] += 1

            def one_pass():
                pr = prpool.tile([P, COLS], bf16, tag="pr")
                cs_ps = pst.tile([1, COLS], f32, tag="cs")
                qt_pe = {}
                if mode == "pe":
                    # single resident group per stream: full PE work, ~no DMA
                    for qsrc, pool, dt_q, tpg, tag in (
                            (qp8, qpool8, f8, TPG8, "q8"),
                            (qp16, qpool16, bf16, TPG16, "q16")):
                        t = pool.tile([P, tpg * E], dt_q, tag=tag)
                        nc.sync.dma_start(t[:], qsrc[0])
                        qt_pe[tag] = t
                qt = None
                for b in range(NB):
                    ps_b = psb.tile([P, P], f32, tag="ps")
                    ncols_b = min((b + 1) * P, S) - b * P
                    if ncols_b < P or mode == "dma":
                        nc.vector.memset(
                            ps_b[:, 0 if mode == "dma" else ncols_b:], NEG)
                    for s in range(b * P, b * P + ncols_b):
                        qsrc, pool, vt, dt_q, tpg, base = stream_of(s)
                        sl = s - base
                        if sl % tpg == 0:
                            if mode == "pe":
                                qt = qt_pe["q8" if s < S8 else "q16"]
                            else:
                                qt = pool.tile([P, tpg * E], dt_q, tag="q")
                                q_dma(qt[:], qsrc[sl // tpg])
                        if mode == "dma":
                            continue
                        j = sl % tpg
                        cl = s - b * P
                        for c in range(CH):
                            nc.tensor.matmul(
                                ps_b[:, cl:cl + 1],
                                qt[:, j * E + c * P: j * E + (c + 1) * P],
                                vt[:, c:c + 1],
                                start=(c == 0), stop=(c == CH - 1))
                    nc.scalar.activation(pr[:, b * P:(b + 1) * P], ps_b[:],
                                         ActF.Exp)
                    nc.tensor.matmul(cs_ps[0:1, b * P:(b + 1) * P],
                                     ones_bf[:], pr[:, b * P:(b + 1) * P],
                                     start=True, stop=True)
                # column sums -> per-row sums -> reciprocals -> per-column
                cs_sb = tpool.tile([1, COLS], f32, tag="cs_sb")
                nc.vector.tensor_copy(cs_sb[:], cs_ps[:])
                csT_ps = pst.tile([P, NB], f32, tag="csT")
                for b in range(NB):
                    nc.tensor.transpose(csT_ps[:, b:b + 1],
                                        cs_sb[0:1, b * P:(b + 1) * P],
                                        iden_sb[0:1, 0:1])
                csT_sb = tpool.tile([P, NB], f32, tag="csT_sb")
                nc.vector.tensor_tensor(out=csT_sb[:], in0=csT_ps[:],
                                        in1=padT_sb[:], op=Alu.subtract)
                rs_ps = pst.tile([P, 1], f32, tag="rs")
                for b in range(NB):
                    nc.tensor.matmul(rs_ps[:], seg_sb[:, b * P:(b + 1) * P],
                                     csT_sb[:, b:b + 1],
                                     start=(b == 0), stop=(b == NB - 1))
                rs_sb = tpool.tile([P, 1], f32, tag="rs_sb")
                nc.vector.tensor_copy(rs_sb[:], rs_ps[:])
                rse = tpool.tile([P, 1], f32, tag="rse")
                # keep unused-row reciprocals finite so 0-weight matmul
                # terms stay 0 instead of 0*inf
                nc.vector.tensor_scalar_add(rse[:], rs_sb[:], 1e-30)
                recip = tpool.tile([P, 1], f32, tag="recip")
                nc.vector.reciprocal(recip[:], rse[:])
                rc_ps = pst.tile([1, COLS], f32, tag="rc")
                nc.tensor.matmul(rc_ps[:], recip[:], segT_sb[:],
                                 start=True, stop=True)
                rc_sb = tpool.tile([1, COLS], f32, tag="rc_sb")
                nc.vector.tensor_copy(rc_sb[:], rc_ps[:])
                bc_ps = pst.tile([P, COLS], f32, tag="bc")
                nc.tensor.matmul(bc_ps[:], ones1_sb[:], rc_sb[:],
                                 start=True, stop=True)
                fin = tpool.tile([P, COLS], f32, tag="fin")
                nc.vector.tensor_tensor(out=fin[:], in0=pr[:], in1=bc_ps[:],
                                        op=Alu.mult)
                # issue the store from the ACT queue: it waits on fin, and on
                # the SP queue that wait would stall the next pass's q loads
                # behind the whole serial softmax tail
                nc.scalar.dma_start(probs[:], fin[:])

            for _rep in range(reps):
                one_pass()

    nc.compile()
    return nc


def kernel(**inputs):
    global LAST_RESULT
    from concourse.bass_utils import run_bass_kernel_spmd

    questions = np.ascontiguousarray(np.asarray(inputs["questions"], np.float32))
    lens = np.asarray(inputs["questions_lens"], np.int32)
    W = np.ascontiguousarray(np.asarray(inputs["W"], np.float32))
    wv = np.ascontiguousarray(np.asarray(inputs["weight_vec"], np.float32))
    B2, L, E_ = questions.shape

    in_maps, cols_meta, G8, G16, NB = _pack(questions, lens, NCORES)
    iden = np.eye(P, dtype=np.float32)
    wvr = np.ascontiguousarray(wv.reshape(CH, P))
    for m in in_maps:
        m["iden"] = iden
        m["wm"] = W
        m["wv"] = wvr
        m["stamp"] = np.zeros((1, 1), np.float32)

    key = (G8, G16, NB)
    if key not in _NC_CACHE:
        _NC_CACHE[key] = _build_nc(G8, G16, NB)
    nc = _NC_CACHE[key]

    # the axon-tunneled device intermittently dies on a first execution
    # (NRT_EXEC_UNIT_UNRECOVERABLE); a straight retry has been observed to
    # succeed, so give it two more chances before giving up
    for attempt in range(3):
        try:
            res = run_bass_kernel_spmd(nc, in_maps, list(range(NCORES)))
            break
        except Exception:
            if attempt == 2:
                raise
    LAST_RESULT = res

    out = np.zeros((B2, L), np.float32)
    for c in range(NCORES):
        pr = res.results[c]["probs"]
        for s, r, t in cols_meta[c]:
            ntok = min(P, int(lens[r]) - t * P)
            out[r, t * P:t * P + ntok] = pr[:ntok, s]
    return out


# revision 56
# speedup vs baseline: 1.2889x; 1.2889x over previous
"""Ragged masked-softmax attention-energy kernel for 8 Trainium2 NeuronCores.

Reference computation (B2=512, L=1024, E=512):
    energy = questions @ W.T + b              [B2, L, E]
    scores = energy @ weight_vec              [B2, L]
    scores[l >= len] = -inf
    out = softmax(scores, axis=1)

Algebraic facts that shape the kernel:
  * (q @ W.T + b) @ wv == q @ (W.T @ wv) + (b . wv); softmax is shift
    invariant so the (b . wv) scalar cancels. Only v = W.T @ wv (a [E]
    vector, computed on device) ever multiplies the big tensor.
  * tokens at positions >= len contribute exactly 0 to the output, so
    only ceil(len/128) 128-token tiles per row need to be loaded at all.

v3 design (PE-matmul dot products, fp8/bf16 hybrid traffic):
  * questions are packed on host TRANSPOSED per 128-token tile: 4 chunks
    of [128 E-rows x 128 tokens]. The per-token dot product with v is
    then 4 accumulating PE matmuls lhsT=[128e,128tok]^T @ v_chunk[128e,1]
    -> PSUM scores in token-major layout. Ablation: this is ~10us/pass
    of PE time vs ~73us of DMA, so the kernel is DMA-bound and bytes are
    everything.
  * rows with len >= 256 are carried in float8_e4m3 (measured worst-case
    softmax abs error 5e-3 at len 256, shrinking with length, vs the
    2e-2 budget -- long rows have small probabilities and exp() spreads
    the error); shorter rows stay bf16 (worst-case 2e-3). fp8 columns
    come first (their own DMA groups), bf16 columns after.
  * No mask tensor: host zero-fills padded tokens, so their score is
    exactly 0 and exp(0)=1; the per-column pad count is subtracted from
    the column sums (exact in fp32). Pad positions of the output are
    never read by the host-side scatter. Columns with no tile at all
    either contribute only to unused rows (segment matrices are 0) or
    are memset to -1e30 so exp gives 0.
  * Per-row softmax denominators via 0/1 segment matmuls on [1,COLS]
    column sums: PE ones-matmul column sums -> transpose -> seg matmul
    row sums -> reciprocal -> segT matmul + K=1 broadcast matmul back to
    [128,COLS] -> one DVE multiply -> DMA out.

Host side does data layout only (bin-packing, zero-fill, transpose,
fp8/bf16 cast, 0/1 indicator matrices); all arithmetic runs on device.
"""

import os
import sys

import numpy as np

if "/opt/trn_rl_repo" not in sys.path:
    sys.path.insert(0, "/opt/trn_rl_repo")

E = 512
P = 128
CH = E // P       # E-chunks per tile (contraction split for the PE)
# fp8 tiles per DMA group (1 MiB; 32-tile/2 MiB groups hard-crash the
# exec unit with NRT_EXEC_UNIT_UNRECOVERABLE) and bf16 tiles per group
TPG8 = int(os.environ.get("TPG8", "16"))
TPG16 = int(os.environ.get("TPG16", "8"))
# which engine DGEs issue the q-group loads: "sync" = all on SP HWDGE,
# "mix" = alternate SP/ACT HWDGEs, "gps" = alternate SP HWDGE / Pool SWDGE
DMAQ = os.environ.get("DMAQ", "sync")
NCORES = 8
NEG = -1.0e30
CUT = 224         # rows with len >= CUT go fp8, shorter rows bf16

_NC_CACHE = {}
LAST_RESULT = None


def _schedule(lens, n_cores):
    """Assign rows to cores (stream-aware LPT, <=128 rows/core).

    The fp8 (len>=CUT) and bf16 streams are balanced independently: the
    per-pass bytes are quantized to whole DMA groups of the max-loaded
    core per stream, so each stream's max matters separately.
    """
    k = [(int(l) + P - 1) // P for l in lens]
    rows_of = [[] for _ in range(n_cores)]
    for stream_rows in ([r for r in range(len(lens)) if lens[r] >= CUT],
                        [r for r in range(len(lens)) if lens[r] < CUT]):
        loads = [0] * n_cores
        for r in sorted(stream_rows, key=lambda r: -k[r]):
            cands = [c for c in range(n_cores) if len(rows_of[c]) < P]
            c = min(cands, key=lambda i: (loads[i], len(rows_of[i])))
            rows_of[c].append(r)
            loads[c] += k[r]
    return rows_of, k


def _pack(questions, lens, n_cores):
    import ml_dtypes

    bf16 = np.dtype(ml_dtypes.bfloat16)
    f8 = np.dtype(ml_dtypes.float8_e4m3)
    B2, L, E_ = questions.shape
    assert E_ == E
    rows_of, k = _schedule(lens, n_cores)
    cols8_of = [[(r, t) for r in rows_of[c] if lens[r] >= CUT
                 for t in range(k[r])] for c in range(n_cores)]
    cols16_of = [[(r, t) for r in rows_of[c] if lens[r] < CUT
                  for t in range(k[r])] for c in range(n_cores)]
    G8 = max(1, max(-(-len(cs) // TPG8) for cs in cols8_of))
    G16 = max(1, max(-(-len(cs) // TPG16) for cs in cols16_of))
    S8, S16 = G8 * TPG8, G16 * TPG16
    S = S8 + S16
    NB = -(-S // P)
    COLS = NB * P
    in_maps = []
    cols_meta = []
    for c in range(n_cores):
        local = {r: i for i, r in enumerate(rows_of[c])}
        # [g, e_lo, j, ch, tok] -> device column base + j*512 + ch*128 + tok
        qp8 = np.zeros((G8, P, TPG8, CH, P), np.float32)
        qp16 = np.zeros((G16, P, TPG16, CH, P), np.float32)
        padT = np.zeros((P, NB), np.float32)
        seg = np.zeros((P, COLS), np.float32)
        segT = np.zeros((P, COLS), np.float32)
        meta = []
        for qp, cols, base, tpg in ((qp8, cols8_of[c], 0, TPG8),
                                    (qp16, cols16_of[c], S8, TPG16)):
            for sl, (r, t) in enumerate(cols):
                g, j = divmod(sl, tpg)
                s = base + sl
                ntok = min(P, int(lens[r]) - t * P)
                blk = questions[r, t * P:t * P + ntok, :].T   # [512, ntok]
                qp[g, :, j, :, :ntok] = blk.reshape(CH, P, ntok).transpose(1, 0, 2)
                b_, m = divmod(s, P)
                padT[m, b_] = float(P - ntok)
                li = local[r]
                seg[m, b_ * P + li] = 1.0
                segT[li, b_ * P + m] = 1.0
                meta.append((s, r, t))
        in_maps.append({"qp8": qp8.reshape(G8, P, TPG8 * E).astype(f8),
                        "qp16": qp16.reshape(G16, P, TPG16 * E).astype(bf16),
                        "padT": padT, "seg": seg, "segT": segT})
        cols_meta.append(meta)
    return in_maps, cols_meta, G8, G16, NB


def _build_nc(G8, G16, NB, reps=1, mode="full"):
    from concourse import bacc, bass, tile

    mybir = bass.mybir
    f32 = mybir.dt.float32
    bf16 = mybir.dt.bfloat16
    f8 = mybir.dt.float8e4
    Alu = mybir.AluOpType
    ActF = mybir.ActivationFunctionType
    S8, S16 = G8 * TPG8, G16 * TPG16
    S = S8 + S16
    COLS = NB * P

    nc = bacc.Bacc("TRN2", target_bir_lowering=False, debug=False,
                   num_devices=NCORES)
    qp8 = nc.declare_dram_parameter("qp8", [G8, P, TPG8 * E], f8,
                                    isOutput=False)
    qp16 = nc.declare_dram_parameter("qp16", [G16, P, TPG16 * E], bf16,
                                     isOutput=False)
    padT = nc.declare_dram_parameter("padT", [P, NB], f32, isOutput=False)
    seg = nc.declare_dram_parameter("seg", [P, COLS], f32, isOutput=False)
    segT = nc.declare_dram_parameter("segT", [P, COLS], f32, isOutput=False)
    iden = nc.declare_dram_parameter("iden", [P, P], f32, isOutput=False)
    wm = nc.declare_dram_parameter("wm", [E, E], f32, isOutput=False)
    wv = nc.declare_dram_parameter("wv", [CH, P], f32, isOutput=False)
    # shape varies with reps/mode so the jax persistent compile cache cannot
    # alias NEFFs of different builds (the BIR is not in the HLO key)
    mid = 1 + ["full", "dma", "pe"].index(mode) \
        + 4 * ["sync", "mix", "gps"].index(DMAQ)
    nc.declare_dram_parameter("stamp", [mid, reps], f32, isOutput=False)
    probs = nc.declare_dram_parameter("probs", [P, COLS], f32, isOutput=True)

    with tile.TileContext(nc) as tc:
        with (
            tc.tile_pool(name="const", bufs=1) as const,
            tc.tile_pool(name="qpool8", bufs=8) as qpool8,
            tc.tile_pool(name="qpool16", bufs=3) as qpool16,
            tc.tile_pool(name="prpool", bufs=2) as prpool,
            tc.tile_pool(name="tpool", bufs=2) as tpool,
            tc.tile_pool(name="psb", bufs=2, space=bass.MemorySpace.PSUM) as psb,
            tc.tile_pool(name="pst", bufs=1, space=bass.MemorySpace.PSUM) as pst,
        ):
            iden_sb = const.tile([P, P], f32, tag="iden")
            nc.sync.dma_start(iden_sb[:], iden[:])
            seg_sb = const.tile([P, COLS], f32, tag="seg")
            nc.sync.dma_start(seg_sb[:], seg[:])
            segT_sb = const.tile([P, COLS], f32, tag="segT")
            nc.sync.dma_start(segT_sb[:], segT[:])
            padT_sb = const.tile([P, NB], f32, tag="padT")
            nc.sync.dma_start(padT_sb[:], padT[:])
            w_sb = const.tile([P, CH * E], f32, tag="wmat")
            for jb in range(CH):
                nc.sync.dma_start(w_sb[:, jb * E:(jb + 1) * E],
                                  wm[jb * P:(jb + 1) * P, :])
            wv4 = const.tile([CH, P], f32, tag="wv4")
            nc.sync.dma_start(wv4[:], wv[:])

            # vT[e_lo, c] = v[c*128+e_lo], v = W.T @ wv, computed on device
            wvT_ps = pst.tile([P, CH], f32, tag="su")
            nc.tensor.transpose(wvT_ps[:], wv4[:], iden_sb[0:CH, 0:CH])
            wvT_sb = const.tile([P, CH], f32, tag="wvT")
            nc.scalar.copy(wvT_sb[:], wvT_ps[:])
            vT_ps = pst.tile([P, CH], f32, tag="su")
            with tc.tile_critical():
                for c in range(CH):
                    for jb in range(CH):
                        nc.tensor.matmul(
                            vT_ps[:, c:c + 1],
                            w_sb[:, jb * E + c * P: jb * E + (c + 1) * P],
                            wvT_sb[:, jb:jb + 1],
                            start=(jb == 0), stop=(jb == CH - 1))
            vT_bf = const.tile([P, CH], bf16, tag="vTbf")
            nc.scalar.copy(vT_bf[:], vT_ps[:])
            vT_f8 = const.tile([P, CH], f8, tag="vTf8")
            nc.scalar.copy(vT_f8[:], vT_ps[:])
            ones_bf = const.tile([P, 1], bf16, tag="ones")
            nc.vector.memset(ones_bf[:], 1.0)
            ones1_sb = const.tile([1, P], f32, tag="ones1")
            nc.vector.memset(ones1_sb[:], 1.0)

            def stream_of(s):
                if s < S8:
                    return qp8, qpool8, vT_f8, f8, TPG8, 0
                return qp16, qpool16, vT_bf, bf16, TPG16, S8

            dma_engines = {"sync": (nc.sync, nc.sync),
                           "mix": (nc.sync, nc.scalar),
                           "gps": (nc.sync, nc.gpsimd)}[DMAQ]
            dma_ctr = [0]

            def q_dma(dst, src):
                dma_engines[dma_ctr[0] % 2].dma_start(dst, src)
                dma_ctr[0] += 1

            def one_pass():
                pr = prpool.tile([P, COLS], bf16, tag="pr")
                cs_ps = pst.tile([1, COLS], f32, tag="cs")
                qt_pe = {}
                if mode == "pe":
                    # single resident group per stream: full PE work, ~no DMA
                    for qsrc, pool, dt_q, tpg, tag in (
                            (qp8, qpool8, f8, TPG8, "q8"),
                            (qp16, qpool16, bf16, TPG16, "q16")):
                        t = pool.tile([P, tpg * E], dt_q, tag=tag)
                        nc.sync.dma_start(t[:], qsrc[0])
                        qt_pe[tag] = t
                qt = None
                for b in range(NB):
                    ps_b = psb.tile([P, P], f32, tag="ps")
                    ncols_b = min((b + 1) * P, S) - b * P
                    if ncols_b < P or mode == "dma":
                        nc.vector.memset(
                            ps_b[:, 0 if mode == "dma" else ncols_b:], NEG)
                    for s in range(b * P, b * P + ncols_b):
                        qsrc, pool, vt, dt_q, tpg, base = stream_of(s)
                        sl = s - base
                        if sl % tpg == 0:
                            if mode == "pe":
                                qt = qt_pe["q8" if s < S8 else "q16"]
                            else:
                                qt = pool.tile([P, tpg * E], dt_q, tag="q")
                                q_dma(qt[:], qsrc[sl // tpg])
                        if mode == "dma":
                            continue
                        j = sl % tpg
                        cl = s - b * P
                        for c in range(CH):
                            nc.tensor.matmul(
                                ps_b[:, cl:cl + 1],
                                qt[:, j * E + c * P: j * E + (c + 1) * P],
                                vt[:, c:c + 1],
                                start=(c == 0), stop=(c == CH - 1))
                    nc.scalar.activation(pr[:, b * P:(b + 1) * P], ps_b[:],
                                         ActF.Exp)
                    nc.tensor.matmul(cs_ps[0:1, b * P:(b + 1) * P],
                                     ones_bf[:], pr[:, b * P:(b + 1) * P],
                                     start=True, stop=True)
                # column sums -> per-row sums -> reciprocals -> per-column
                cs_sb = tpool.tile([1, COLS], f32, tag="cs_sb")
                nc.vector.tensor_copy(cs_sb[:], cs_ps[:])
                csT_ps = pst.tile([P, NB], f32, tag="csT")
                for b in range(NB):
                    nc.tensor.transpose(csT_ps[:, b:b + 1],
                                        cs_sb[0:1, b * P:(b + 1) * P],
                                        iden_sb[0:1, 0:1])
                csT_sb = tpool.tile([P, NB], f32, tag="csT_sb")
                nc.vector.tensor_tensor(out=csT_sb[:], in0=csT_ps[:],
                                        in1=padT_sb[:], op=Alu.subtract)
                rs_ps = pst.tile([P, 1], f32, tag="rs")
                for b in range(NB):
                    nc.tensor.matmul(rs_ps[:], seg_sb[:, b * P:(b + 1) * P],
                                     csT_sb[:, b:b + 1],
                                     start=(b == 0), stop=(b == NB - 1))
                rs_sb = tpool.tile([P, 1], f32, tag="rs_sb")
                nc.vector.tensor_copy(rs_sb[:], rs_ps[:])
                rse = tpool.tile([P, 1], f32, tag="rse")
                # keep unused-row reciprocals finite so 0-weight matmul
                # terms stay 0 instead of 0*inf
                nc.vector.tensor_scalar_add(rse[:], rs_sb[:], 1e-30)
                recip = tpool.tile([P, 1], f32, tag="recip")
                nc.vector.reciprocal(recip[:], rse[:])
                rc_ps = pst.tile([1, COLS], f32, tag="rc")
                nc.tensor.matmul(rc_ps[:], recip[:], segT_sb[:],
                                 start=True, stop=True)
                rc_sb = tpool.tile([1, COLS], f32, tag="rc_sb")
                nc.vector.tensor_copy(rc_sb[:], rc_ps[:])
                bc_ps = pst.tile([P, COLS], f32, tag="bc")
                nc.tensor.matmul(bc_ps[:], ones1_sb[:], rc_sb[:],
                                 start=True, stop=True)
                fin = tpool.tile([P, COLS], f32, tag="fin")
                nc.vector.tensor_tensor(out=fin[:], in0=pr[:], in1=bc_ps[:],
                                        op=Alu.mult)
                # issue the store from the ACT queue: it waits on fin, and on
                # the SP queue that wait would stall the next pass's q loads
                # behind the whole serial softmax tail
                nc.scalar.dma_start(probs[:], fin[:])

            for _rep in range(reps):
                one_pass()

    nc.compile()
    return nc


def kernel(**inputs):
    global LAST_RESULT
    from concourse.bass_utils import run_bass_kernel_spmd

    questions = np.ascontiguousarray(np.asarray(inputs["questions"], np.float32))
    lens = np.asarray(inputs["questions_lens"], np.int32)
    W = np.ascontiguousarray(np.asarray(inputs["W"], np.float32))
    wv = np.ascontiguousarray(np.asarray(inputs["weight_vec"], np.float32))
    B2, L, E_ = questions.shape

    in_maps, cols_meta, G8, G16, NB = _pack(questions, lens, NCORES)
    iden = np.eye(P, dtype=np.float32)
    wvr = np.ascontiguousarray(wv.reshape(CH, P))
    for m in in_maps:
        m["iden"] = iden
        m["wm"] = W
        m["wv"] = wvr
        m["stamp"] = np.zeros((1, 1), np.float32)

    key = (G8, G16, NB)
    if key not in _NC_CACHE:
        _NC_CACHE[key] = _build_nc(G8, G16, NB)
    nc = _NC_CACHE[key]

    # the axon-tunneled device intermittently dies on a first execution
    # (NRT_EXEC_UNIT_UNRECOVERABLE); a straight retry has been observed to
    # succeed, so give it two more chances before giving up
    for attempt in range(3):
        try:
            res = run_bass_kernel_spmd(nc, in_maps, list(range(NCORES)))
            break
        except Exception:
            if attempt == 2:
                raise
    LAST_RESULT = res

    out = np.zeros((B2, L), np.float32)
    for c in range(NCORES):
        pr = res.results[c]["probs"]
        for s, r, t in cols_meta[c]:
            ntok = min(P, int(lens[r]) - t * P)
            out[r, t * P:t * P + ntok] = pr[:ntok, s]
    return out


# revision 57
# speedup vs baseline: 1.5672x; 1.2159x over previous
"""Ragged masked-softmax attention-energy kernel for 8 Trainium2 NeuronCores.

Reference computation (B2=512, L=1024, E=512):
    energy = questions @ W.T + b              [B2, L, E]
    scores = energy @ weight_vec              [B2, L]
    scores[l >= len] = -inf
    out = softmax(scores, axis=1)

Algebraic facts that shape the kernel:
  * (q @ W.T + b) @ wv == q @ (W.T @ wv) + (b . wv); softmax is shift
    invariant so the (b . wv) scalar cancels. Only v = W.T @ wv (a [E]
    vector, computed on device) ever multiplies the big tensor.
  * tokens at positions >= len contribute exactly 0 to the output, so
    only ceil(len/128) 128-token tiles per row need to be loaded at all.

v3 design (PE-matmul dot products, fp8/bf16 hybrid traffic):
  * questions are packed on host TRANSPOSED per 128-token tile: 4 chunks
    of [128 E-rows x 128 tokens]. The per-token dot product with v is
    then 4 accumulating PE matmuls lhsT=[128e,128tok]^T @ v_chunk[128e,1]
    -> PSUM scores in token-major layout. Ablation: this is ~10us/pass
    of PE time vs ~73us of DMA, so the kernel is DMA-bound and bytes are
    everything.
  * rows with len >= CUT=224 are carried in float8_e4m3 (measured
    worst-case softmax abs error 5e-3 at len 256, shrinking with length
    and similar at 224, vs the
    2e-2 budget -- long rows have small probabilities and exp() spreads
    the error); shorter rows stay bf16 (worst-case 2e-3). fp8 columns
    come first (their own DMA groups), bf16 columns after.
  * No mask tensor: host zero-fills padded tokens, so their score is
    exactly 0 and exp(0)=1; the per-column pad count is subtracted from
    the column sums (exact in fp32). Pad positions of the output are
    never read by the host-side scatter. Columns with no tile at all
    either contribute only to unused rows (segment matrices are 0) or
    are memset to -1e30 so exp gives 0.
  * Per-row softmax denominators via 0/1 segment matmuls on [1,COLS]
    column sums: PE ones-matmul column sums -> transpose -> seg matmul
    row sums -> reciprocal -> segT matmul + K=1 broadcast matmul back to
    [128,COLS] -> one DVE multiply -> DMA out.

Host side does data layout only (bin-packing, zero-fill, transpose,
fp8/bf16 cast, 0/1 indicator matrices); all arithmetic runs on device.
"""

import os
import sys

import numpy as np

if "/opt/trn_rl_repo" not in sys.path:
    sys.path.insert(0, "/opt/trn_rl_repo")

E = 512
P = 128
CH = E // P       # E-chunks per tile (contraction split for the PE)
# fp8 tiles per DMA group (1 MiB; 32-tile/2 MiB groups hard-crash the
# exec unit with NRT_EXEC_UNIT_UNRECOVERABLE) and bf16 tiles per group
TPG8 = int(os.environ.get("TPG8", "16"))
TPG16 = int(os.environ.get("TPG16", "8"))
# which engine DGEs issue the q-group loads: "sync" = all on SP HWDGE,
# "mix" = alternate SP/ACT HWDGEs, "gps" = alternate SP HWDGE / Pool SWDGE
DMAQ = os.environ.get("DMAQ", "sync")
NCORES = 8
NEG = -1.0e30
CUT = 224         # rows with len >= CUT go fp8, shorter rows bf16

_NC_CACHE = {}
LAST_RESULT = None


def _schedule(lens, n_cores):
    """Assign rows to cores (stream-aware LPT, <=128 rows/core).

    The fp8 (len>=CUT) and bf16 streams are balanced independently: the
    per-pass bytes are quantized to whole DMA groups of the max-loaded
    core per stream, so each stream's max matters separately.
    """
    k = [(int(l) + P - 1) // P for l in lens]
    rows_of = [[] for _ in range(n_cores)]
    for stream_rows in ([r for r in range(len(lens)) if lens[r] >= CUT],
                        [r for r in range(len(lens)) if lens[r] < CUT]):
        loads = [0] * n_cores
        for r in sorted(stream_rows, key=lambda r: -k[r]):
            cands = [c for c in range(n_cores) if len(rows_of[c]) < P]
            c = min(cands, key=lambda i: (loads[i], len(rows_of[i])))
            rows_of[c].append(r)
            loads[c] += k[r]
    return rows_of, k


def _pack(questions, lens, n_cores):
    import ml_dtypes

    bf16 = np.dtype(ml_dtypes.bfloat16)
    f8 = np.dtype(ml_dtypes.float8_e4m3)
    B2, L, E_ = questions.shape
    assert E_ == E
    rows_of, k = _schedule(lens, n_cores)
    cols8_of = [[(r, t) for r in rows_of[c] if lens[r] >= CUT
                 for t in range(k[r])] for c in range(n_cores)]
    cols16_of = [[(r, t) for r in rows_of[c] if lens[r] < CUT
                  for t in range(k[r])] for c in range(n_cores)]
    G8 = max(1, max(-(-len(cs) // TPG8) for cs in cols8_of))
    G16 = max(1, max(-(-len(cs) // TPG16) for cs in cols16_of))
    S8, S16 = G8 * TPG8, G16 * TPG16
    S = S8 + S16
    NB = -(-S // P)
    COLS = NB * P
    in_maps = []
    cols_meta = []
    for c in range(n_cores):
        local = {r: i for i, r in enumerate(rows_of[c])}
        # [g, e_lo, j, ch, tok] -> device column base + j*512 + ch*128 + tok
        qp8 = np.zeros((G8, P, TPG8, CH, P), np.float32)
        qp16 = np.zeros((G16, P, TPG16, CH, P), np.float32)
        padT = np.zeros((P, NB), np.float32)
        seg = np.zeros((P, COLS), np.float32)
        segT = np.zeros((P, COLS), np.float32)
        meta = []
        for qp, cols, base, tpg in ((qp8, cols8_of[c], 0, TPG8),
                                    (qp16, cols16_of[c], S8, TPG16)):
            for sl, (r, t) in enumerate(cols):
                g, j = divmod(sl, tpg)
                s = base + sl
                ntok = min(P, int(lens[r]) - t * P)
                blk = questions[r, t * P:t * P + ntok, :].T   # [512, ntok]
                qp[g, :, j, :, :ntok] = blk.reshape(CH, P, ntok).transpose(1, 0, 2)
                b_, m = divmod(s, P)
                padT[m, b_] = float(P - ntok)
                li = local[r]
                seg[m, b_ * P + li] = 1.0
                segT[li, b_ * P + m] = 1.0
                meta.append((s, r, t))
        in_maps.append({"qp8": qp8.reshape(G8, P, TPG8 * E).astype(f8),
                        "qp16": qp16.reshape(G16, P, TPG16 * E).astype(bf16),
                        "padT": padT, "seg": seg, "segT": segT})
        cols_meta.append(meta)
    return in_maps, cols_meta, G8, G16, NB


def _build_nc(G8, G16, NB, reps=1, mode="full"):
    from concourse import bacc, bass, tile

    mybir = bass.mybir
    f32 = mybir.dt.float32
    bf16 = mybir.dt.bfloat16
    f8 = mybir.dt.float8e4
    Alu = mybir.AluOpType
    ActF = mybir.ActivationFunctionType
    S8, S16 = G8 * TPG8, G16 * TPG16
    S = S8 + S16
    COLS = NB * P

    nc = bacc.Bacc("TRN2", target_bir_lowering=False, debug=False,
                   num_devices=NCORES)
    qp8 = nc.declare_dram_parameter("qp8", [G8, P, TPG8 * E], f8,
                                    isOutput=False)
    qp16 = nc.declare_dram_parameter("qp16", [G16, P, TPG16 * E], bf16,
                                     isOutput=False)
    padT = nc.declare_dram_parameter("padT", [P, NB], f32, isOutput=False)
    seg = nc.declare_dram_parameter("seg", [P, COLS], f32, isOutput=False)
    segT = nc.declare_dram_parameter("segT", [P, COLS], f32, isOutput=False)
    iden = nc.declare_dram_parameter("iden", [P, P], f32, isOutput=False)
    wm = nc.declare_dram_parameter("wm", [E, E], f32, isOutput=False)
    wv = nc.declare_dram_parameter("wv", [CH, P], f32, isOutput=False)
    # shape varies with reps/mode so the jax persistent compile cache cannot
    # alias NEFFs of different builds (the BIR is not in the HLO key)
    mid = 1 + ["full", "dma", "pe"].index(mode) \
        + 4 * ["sync", "mix", "gps"].index(DMAQ)
    nc.declare_dram_parameter("stamp", [mid, reps], f32, isOutput=False)
    probs = nc.declare_dram_parameter("probs", [P, COLS], f32, isOutput=True)

    with tile.TileContext(nc) as tc:
        with (
            tc.tile_pool(name="const", bufs=1) as const,
            tc.tile_pool(name="qpool8", bufs=8) as qpool8,
            tc.tile_pool(name="qpool16", bufs=3) as qpool16,
            tc.tile_pool(name="prpool", bufs=2) as prpool,
            tc.tile_pool(name="tpool", bufs=2) as tpool,
            tc.tile_pool(name="psb", bufs=2, space=bass.MemorySpace.PSUM) as psb,
            tc.tile_pool(name="pst", bufs=1, space=bass.MemorySpace.PSUM) as pst,
        ):
            iden_sb = const.tile([P, P], f32, tag="iden")
            nc.sync.dma_start(iden_sb[:], iden[:])
            seg_sb = const.tile([P, COLS], f32, tag="seg")
            nc.sync.dma_start(seg_sb[:], seg[:])
            segT_sb = const.tile([P, COLS], f32, tag="segT")
            nc.sync.dma_start(segT_sb[:], segT[:])
            padT_sb = const.tile([P, NB], f32, tag="padT")
            nc.sync.dma_start(padT_sb[:], padT[:])
            w_sb = const.tile([P, CH * E], f32, tag="wmat")
            for jb in range(CH):
                nc.sync.dma_start(w_sb[:, jb * E:(jb + 1) * E],
                                  wm[jb * P:(jb + 1) * P, :])
            wv4 = const.tile([CH, P], f32, tag="wv4")
            nc.sync.dma_start(wv4[:], wv[:])

            # vT[e_lo, c] = v[c*128+e_lo], v = W.T @ wv, computed on device
            wvT_ps = pst.tile([P, CH], f32, tag="su")
            nc.tensor.transpose(wvT_ps[:], wv4[:], iden_sb[0:CH, 0:CH])
            wvT_sb = const.tile([P, CH], f32, tag="wvT")
            nc.scalar.copy(wvT_sb[:], wvT_ps[:])
            vT_ps = pst.tile([P, CH], f32, tag="su")
            with tc.tile_critical():
                for c in range(CH):
                    for jb in range(CH):
                        nc.tensor.matmul(
                            vT_ps[:, c:c + 1],
                            w_sb[:, jb * E + c * P: jb * E + (c + 1) * P],
                            wvT_sb[:, jb:jb + 1],
                            start=(jb == 0), stop=(jb == CH - 1))
            vT_bf = const.tile([P, CH], bf16, tag="vTbf")
            nc.scalar.copy(vT_bf[:], vT_ps[:])
            vT_f8 = const.tile([P, CH], f8, tag="vTf8")
            nc.scalar.copy(vT_f8[:], vT_ps[:])
            ones_bf = const.tile([P, 1], bf16, tag="ones")
            nc.vector.memset(ones_bf[:], 1.0)
            ones1_sb = const.tile([1, P], f32, tag="ones1")
            nc.vector.memset(ones1_sb[:], 1.0)

            def stream_of(s):
                if s < S8:
                    return qp8, qpool8, vT_f8, f8, TPG8, 0
                return qp16, qpool16, vT_bf, bf16, TPG16, S8

            dma_engines = {"sync": (nc.sync, nc.sync),
                           "mix": (nc.sync, nc.scalar),
                           "gps": (nc.sync, nc.gpsimd)}[DMAQ]
            dma_ctr = [0]

            def q_dma(dst, src):
                dma_engines[dma_ctr[0] % 2].dma_start(dst, src)
                dma_ctr[0] += 1

            def one_pass():
                pr = prpool.tile([P, COLS], bf16, tag="pr")
                cs_ps = pst.tile([1, COLS], f32, tag="cs")
                qt_pe = {}
                if mode == "pe":
                    # single resident group per stream: full PE work, ~no DMA
                    for qsrc, pool, dt_q, tpg, tag in (
                            (qp8, qpool8, f8, TPG8, "q8"),
                            (qp16, qpool16, bf16, TPG16, "q16")):
                        t = pool.tile([P, tpg * E], dt_q, tag=tag)
                        nc.sync.dma_start(t[:], qsrc[0])
                        qt_pe[tag] = t
                qt = None
                for b in range(NB):
                    ps_b = psb.tile([P, P], f32, tag="ps")
                    ncols_b = min((b + 1) * P, S) - b * P
                    if ncols_b < P or mode == "dma":
                        nc.vector.memset(
                            ps_b[:, 0 if mode == "dma" else ncols_b:], NEG)
                    for s in range(b * P, b * P + ncols_b):
                        qsrc, pool, vt, dt_q, tpg, base = stream_of(s)
                        sl = s - base
                        if sl % tpg == 0:
                            if mode == "pe":
                                qt = qt_pe["q8" if s < S8 else "q16"]
                            else:
                                qt = pool.tile([P, tpg * E], dt_q, tag="q")
                                q_dma(qt[:], qsrc[sl // tpg])
                        if mode == "dma":
                            continue
                        j = sl % tpg
                        cl = s - b * P
                        for c in range(CH):
                            nc.tensor.matmul(
                                ps_b[:, cl:cl + 1],
                                qt[:, j * E + c * P: j * E + (c + 1) * P],
                                vt[:, c:c + 1],
                                start=(c == 0), stop=(c == CH - 1))
                    nc.scalar.activation(pr[:, b * P:(b + 1) * P], ps_b[:],
                                         ActF.Exp)
                    nc.tensor.matmul(cs_ps[0:1, b * P:(b + 1) * P],
                                     ones_bf[:], pr[:, b * P:(b + 1) * P],
                                     start=True, stop=True)
                # column sums -> per-row sums -> reciprocals -> per-column
                cs_sb = tpool.tile([1, COLS], f32, tag="cs_sb")
                nc.vector.tensor_copy(cs_sb[:], cs_ps[:])
                csT_ps = pst.tile([P, NB], f32, tag="csT")
                for b in range(NB):
                    nc.tensor.transpose(csT_ps[:, b:b + 1],
                                        cs_sb[0:1, b * P:(b + 1) * P],
                                        iden_sb[0:1, 0:1])
                csT_sb = tpool.tile([P, NB], f32, tag="csT_sb")
                nc.vector.tensor_tensor(out=csT_sb[:], in0=csT_ps[:],
                                        in1=padT_sb[:], op=Alu.subtract)
                rs_ps = pst.tile([P, 1], f32, tag="rs")
                for b in range(NB):
                    nc.tensor.matmul(rs_ps[:], seg_sb[:, b * P:(b + 1) * P],
                                     csT_sb[:, b:b + 1],
                                     start=(b == 0), stop=(b == NB - 1))
                rs_sb = tpool.tile([P, 1], f32, tag="rs_sb")
                nc.vector.tensor_copy(rs_sb[:], rs_ps[:])
                rse = tpool.tile([P, 1], f32, tag="rse")
                # keep unused-row reciprocals finite so 0-weight matmul
                # terms stay 0 instead of 0*inf
                nc.vector.tensor_scalar_add(rse[:], rs_sb[:], 1e-30)
                recip = tpool.tile([P, 1], f32, tag="recip")
                nc.vector.reciprocal(recip[:], rse[:])
                rc_ps = pst.tile([1, COLS], f32, tag="rc")
                nc.tensor.matmul(rc_ps[:], recip[:], segT_sb[:],
                                 start=True, stop=True)
                rc_sb = tpool.tile([1, COLS], f32, tag="rc_sb")
                nc.vector.tensor_copy(rc_sb[:], rc_ps[:])
                bc_ps = pst.tile([P, COLS], f32, tag="bc")
                nc.tensor.matmul(bc_ps[:], ones1_sb[:], rc_sb[:],
                                 start=True, stop=True)
                fin = tpool.tile([P, COLS], f32, tag="fin")
                nc.vector.tensor_tensor(out=fin[:], in0=pr[:], in1=bc_ps[:],
                                        op=Alu.mult)
                # issue the store from the ACT queue: it waits on fin, and on
                # the SP queue that wait would stall the next pass's q loads
                # behind the whole serial softmax tail
                nc.scalar.dma_start(probs[:], fin[:])

            for _rep in range(reps):
                one_pass()

    nc.compile()
    return nc


def kernel(**inputs):
    global LAST_RESULT
    from concourse.bass_utils import run_bass_kernel_spmd

    questions = np.ascontiguousarray(np.asarray(inputs["questions"], np.float32))
    lens = np.asarray(inputs["questions_lens"], np.int32)
    W = np.ascontiguousarray(np.asarray(inputs["W"], np.float32))
    wv = np.ascontiguousarray(np.asarray(inputs["weight_vec"], np.float32))
    B2, L, E_ = questions.shape

    in_maps, cols_meta, G8, G16, NB = _pack(questions, lens, NCORES)
    iden = np.eye(P, dtype=np.float32)
    wvr = np.ascontiguousarray(wv.reshape(CH, P))
    for m in in_maps:
        m["iden"] = iden
        m["wm"] = W
        m["wv"] = wvr
        m["stamp"] = np.zeros((1, 1), np.float32)

    key = (G8, G16, NB)
    if key not in _NC_CACHE:
        _NC_CACHE[key] = _build_nc(G8, G16, NB)
    nc = _NC_CACHE[key]

    # the axon-tunneled device intermittently dies on a first execution
    # (NRT_EXEC_UNIT_UNRECOVERABLE); a straight retry has been observed to
    # succeed, so give it two more chances before giving up
    for attempt in range(3):
        try:
            res = run_bass_kernel_spmd(nc, in_maps, list(range(NCORES)))
            break
        except Exception:
            if attempt == 2:
                raise
    LAST_RESULT = res

    out = np.zeros((B2, L), np.float32)
    for c in range(NCORES):
        pr = res.results[c]["probs"]
        for s, r, t in cols_meta[c]:
            ntok = min(P, int(lens[r]) - t * P)
            out[r, t * P:t * P + ntok] = pr[:ntok, s]
    return out
